# revision 1
# baseline (speedup 1.0000x reference)
"""Trainium2 Bass kernel for nn_BasicModel_28724741276284.

Computes, for E=200000 candidates with S=8 node indices + 1 hyperedge index:
  star   = sigmoid(min_s <hyperedge_emb[h], node_emb[X[:,s]]>)
  clique = sigmoid(min_{s,t} <node_emb[X[:,s]], node_emb[X[:,t]]>)
(out [E, 2] fp32).  sigmoid is monotonic, so min(sigmoid(x)) == sigmoid(min(x)).

Sharding: data-parallel over candidates across 8 NeuronCores; the two
embedding tables are concatenated into one [150000, 64] fp32 table and
replicated to every core.

Per-core dataflow (EC=25074 candidates = 199 tiles x 126):
  1. SWDGE indirect DMAs gather the 9 referenced table rows per candidate
     into SBUF [126 cand-partitions, 9*64].  HW semantics allow exactly one
     index per partition per call (a contiguous run each), so this is 9
     calls per tile -- the kernel's bottleneck (~1µs fixed cost per call).
  2. 9 PE transposes (one per row-slot) -> feature-major [64, 126] blocks in
     PSUM, then DVE copies interleave them into tall [64, 9*126] laid out
     (group, s, cand) so each gram group is a contiguous [64, 126] slice
     with s-major columns.
  3. 9 gram matmuls, one per group of 14 candidates: lhsT = rhs =
     tall[:, 126g:126(g+1)] -> PSUM [126,126] holding all pairwise dots of
     each candidate's 9 rows.
  4. mask+row-min (tensor_add of +1e30 off-candidate-block / hyperedge-col
     mask, then tensor_reduce min) -> rm[126,1].  Rows 14s+c (s<8) are
     clique row-mins; rows 112+c are the star mins.  (The fused
     tensor_tensor_reduce op crashes this runtime -- see USE_TTR.)
  5. rm columns staged [126, 1792]; 14 batched PE transposes + an 8-way
     tensor_tensor min tree over s + sigmoid -> packed output [128, 392]
     per core; host applies the inverse permutation (pure layout, no math).
"""

import numpy as np

D = 64
S9 = 9          # 8 node rows + 1 hyperedge row per candidate
GC = 14         # candidates per gram group
NG = 9          # groups per tile
TILE = GC * NG  # 126 candidates per tile
M9 = S9 * GC    # 126 stacked rows per group
NT = 199        # tiles per core
EC = TILE * NT  # 25074 candidates per core
NCORES = 8
EPAD = EC * NCORES  # 200592
SCOLS = NT * NG     # 1791 rm-stage columns
CHUNK = 128
NCHUNK = (SCOLS + CHUNK - 1) // CHUNK  # 14
N_NODES = 100000
N_HYP = 50000
E = 200000
SUPER = 4       # tiles per gather

BIG = 1.0e30
USE_TTR = False            # fused tensor_tensor_reduce for mask+row-min
USE_STRIDED_REDUCE = False  # strided-inner-dim reduce in the finishing pass

_cached = {}


def _build_nc():
    import concourse.bass as bass
    import concourse.tile as tile
    from concourse import bacc, mybir
    from contextlib import ExitStack

    f32 = mybir.dt.float32
    i32 = mybir.dt.int32

    nc = bacc.Bacc(trn_type="TRN2", target_bir_lowering=False, debug=False)

    # Version-tag input: its shape encodes a hash of this source file, so any
    # kernel change yields a different HLO and cannot hit a stale NEFF in
    # /root/.neuron-compile-cache (the cache keys on HLO, not on the BIR the
    # neuronx_cc_hook injects).
    import hashlib

    try:
        src = open(__file__, "rb").read()
    except Exception:
        src = b"?"
    hh = hashlib.md5(src).digest()
    _cached["vertag_shape"] = (1 + hh[0], 1 + hh[1])
    nc.dram_tensor("vertag", list(_cached["vertag_shape"]), f32, kind="ExternalInput")

    table = nc.dram_tensor("table", [N_NODES + N_HYP, D], f32, kind="ExternalInput").ap()
    idx = nc.dram_tensor("idx", [TILE, SCOLS], i32, kind="ExternalInput").ap()
    ident = nc.dram_tensor("ident", [128, 128], f32, kind="ExternalInput").ap()
    maskc = nc.dram_tensor("maskc", [M9, M9], f32, kind="ExternalInput").ap()
    outfin = nc.dram_tensor("outfin", [128, 2 * NCHUNK * GC], f32, kind="ExternalOutput").ap()

    with tile.TileContext(nc) as tc, ExitStack() as ctx:
        const_pool = ctx.enter_context(tc.tile_pool(name="consts", bufs=1))
        emb_pool = ctx.enter_context(tc.tile_pool(name="emb", bufs=2))
        tall_pool = ctx.enter_context(tc.tile_pool(name="tall", bufs=2))
        scratch_pool = ctx.enter_context(tc.tile_pool(name="scratch", bufs=2))
        stage_pool = ctx.enter_context(tc.tile_pool(name="stage", bufs=1))
        fin_pool = ctx.enter_context(tc.tile_pool(name="fin", bufs=1))
        tp_pool = ctx.enter_context(tc.tile_pool(name="tpsum", bufs=1, space="PSUM"))
        gram_pool = ctx.enter_context(tc.tile_pool(name="gram", bufs=1, space="PSUM"))
        fps_pool = ctx.enter_context(tc.tile_pool(name="fpsum", bufs=1, space="PSUM"))

        # --- constants / staging ---
        idx_sb = const_pool.tile([TILE, SCOLS], i32)
        nc.sync.dma_start(out=idx_sb[:], in_=idx[:])
        ident_sb = const_pool.tile([128, 128], f32)
        nc.sync.dma_start(out=ident_sb[:], in_=ident[:])
        mask_sb = const_pool.tile([M9, M9], f32)
        nc.sync.dma_start(out=mask_sb[:], in_=maskc[:])

        rm_stage = stage_pool.tile([M9, NCHUNK * CHUNK], f32)
        nc.vector.memset(rm_stage[:, SCOLS:], 0.0)

        if True:
            for t in range(NT):
                # HW indirect-DMA semantics: ONE index per partition per call,
                # a contiguous run per partition -> 9 calls per tile.
                emb = emb_pool.tile([TILE, S9 * D], f32, tag="emb")
                for s in range(S9):
                    nc.gpsimd.indirect_dma_start(
                        out=emb[:, D * s : D * (s + 1)],
                        out_offset=None,
                        in_=table[:, :],
                        in_offset=bass.IndirectOffsetOnAxis(
                            ap=idx_sb[:, S9 * t + s : S9 * t + s + 1], axis=0
                        ),
                    )
                et = emb[:, :]

                # --- transposes: 9 x [126, 64] -> [64, 126] packed in PSUM ---
                tpA = tp_pool.tile([64, 504], f32, tag="tpA")
                tpB = tp_pool.tile([64, 504], f32, tag="tpB")
                tpC = tp_pool.tile([64, 126], f32, tag="tpC")
                for s in range(S9):
                    if s < 4:
                        dst = tpA[:, 126 * s : 126 * (s + 1)]
                    elif s < 8:
                        dst = tpB[:, 126 * (s - 4) : 126 * (s - 3)]
                    else:
                        dst = tpC[:, :]
                    nc.tensor.transpose(
                        out=dst,
                        in_=et[:, D * s : D * (s + 1)],
                        identity=ident_sb[:TILE, :TILE],
                    )
                # tall layout: col = 126*g + 14*s + c  (group-major, s-major
                # within group) so each gram group is a contiguous [64, 126]
                # slice; the (s,g,c) interleave happens in the copy out-APs.
                tall = tall_pool.tile([64, S9 * TILE], f32, tag="tall")
                tw = tall[:].rearrange("p (g s c) -> p s g c", g=NG, s=S9)
                # one 3-D-AP copy per row-slot s (conservative: <=2 free dims
                # per operand side beyond what walrus/DVE verifiably support)
                for s in range(S9):
                    if s < 4:
                        src_s = tpA[:, 126 * s : 126 * (s + 1)]
                    elif s < 8:
                        src_s = tpB[:, 126 * (s - 4) : 126 * (s - 3)]
                    else:
                        src_s = tpC[:, :]
                    nc.vector.tensor_copy(
                        out=tw[:, s],
                        in_=src_s.rearrange("p (g c) -> p g c", g=NG),
                    )

                # --- gram matmuls: one per group of 14 candidates ---
                gA = gram_pool.tile([M9, 504], f32, tag="gA")
                gB = gram_pool.tile([M9, 504], f32, tag="gB")
                gC = gram_pool.tile([M9, 126], f32, tag="gC")
                for g in range(NG):
                    stacked = tall[:, 126 * g : 126 * (g + 1)]
                    if g < 4:
                        dst = gA[:, 126 * g : 126 * (g + 1)]
                    elif g < 8:
                        dst = gB[:, 126 * (g - 4) : 126 * (g - 3)]
                    else:
                        dst = gC[:, :]
                    nc.tensor.matmul(out=dst, lhsT=stacked, rhs=stacked)

                # --- fused mask + row-min per group ---
                for g in range(NG):
                    if g < 4:
                        src = gA[:, 126 * g : 126 * (g + 1)]
                    elif g < 8:
                        src = gB[:, 126 * (g - 4) : 126 * (g - 3)]
                    else:
                        src = gC[:, :]
                    scr = scratch_pool.tile([M9, M9], f32, tag="scr")
                    if USE_TTR:
                        nc.vector.tensor_tensor_reduce(
                            out=scr[:],
                            in0=src,
                            in1=mask_sb[:],
                            scale=1.0,
                            scalar=BIG,
                            op0=mybir.AluOpType.add,
                            op1=mybir.AluOpType.min,
                            accum_out=rm_stage[:, NG * t + g : NG * t + g + 1],
                        )
                    else:
                        nc.vector.tensor_add(scr[:], src, mask_sb[:])
                        nc.vector.tensor_reduce(
                            out=rm_stage[:, NG * t + g : NG * t + g + 1],
                            in_=scr[:],
                            axis=mybir.AxisListType.XY,
                            op=mybir.AluOpType.min,
                        )

        # --- finishing: transpose rm_stage chunks, min over s, sigmoid ---
        fin = fin_pool.tile([128, 2 * NCHUNK * GC], f32)
        for k in range(NCHUNK):
            tk = fps_pool.tile([128, M9], f32, tag="tk")
            nc.tensor.transpose(
                out=tk[:],
                in_=rm_stage[:, CHUNK * k : CHUNK * (k + 1)],
                identity=ident_sb[:M9, :M9],
            )
            if USE_STRIDED_REDUCE:
                tkv = tk[:].rearrange("q (s c) -> q c s", s=S9)
                nc.vector.tensor_reduce(
                    out=fin[:, GC * k : GC * (k + 1)],
                    in_=tkv[:, :, 0:8],
                    axis=mybir.AxisListType.X,
                    op=mybir.AluOpType.min,
                )
            else:
                dst = fin[:, GC * k : GC * (k + 1)]
                nc.vector.tensor_copy(out=dst, in_=tk[:, 0:GC])
                for s in range(1, 8):
                    nc.vector.tensor_tensor(
                        out=dst,
                        in0=dst,
                        in1=tk[:, GC * s : GC * (s + 1)],
                        op=mybir.AluOpType.min,
                    )
            nc.vector.tensor_copy(
                out=fin[:, NCHUNK * GC + GC * k : NCHUNK * GC + GC * (k + 1)],
                in_=tk[:, 112:126],
            )
        fin2 = fin_pool.tile([128, 2 * NCHUNK * GC], f32)
        nc.scalar.activation(
            out=fin2[:], in_=fin[:], func=mybir.ActivationFunctionType.Sigmoid
        )
        nc.sync.dma_start(out=outfin[:], in_=fin2[:])

    nc.compile()
    return nc


def _host_inputs(node_emb, hyperedge_emb, h, X):
    table = np.concatenate(
        [np.asarray(node_emb, np.float32), np.asarray(hyperedge_emb, np.float32)],
        axis=0,
    )
    table = np.ascontiguousarray(table)
    h32 = np.asarray(h, np.int64).astype(np.int32) + N_NODES
    X32 = np.asarray(X, np.int64).astype(np.int32)
    idx9 = np.concatenate([X32, h32[:, None]], axis=1)  # [E, 9]
    pad = np.zeros((EPAD - E, S9), np.int32)
    idx9 = np.concatenate([idx9, pad], axis=0)  # [EPAD, 9]

    ident = np.eye(128, dtype=np.float32)
    p = np.arange(M9)
    j = np.arange(M9)
    mask = np.where((p[:, None] % GC == j[None, :] % GC) & (j[None, :] < 112), 0.0, BIG)
    mask = mask.astype(np.float32)

    vertag = np.zeros(_cached.get("vertag_shape", (1, 1)), np.float32)
    per_core = []
    for r in range(NCORES):
        shard = idx9[r * EC : (r + 1) * EC]  # [EC, 9]
        idx_t = shard.reshape(NT, TILE, S9).transpose(1, 0, 2).reshape(TILE, SCOLS)
        per_core.append(
            {
                "table": table,
                "idx": np.ascontiguousarray(idx_t),
                "ident": ident,
                "maskc": mask,
                "vertag": vertag,
            }
        )
    return per_core


def _decode(outs):
    """outs: list of per-core [128, 392] arrays -> [E, 2] fp32."""
    t = np.arange(NT)[:, None, None]
    g = np.arange(NG)[None, :, None]
    c = np.arange(GC)[None, None, :]
    sc = NG * t + g  # stage column
    k = sc // CHUNK
    q = sc % CHUNK
    res = np.empty((EPAD, 2), np.float32)
    for r, of in enumerate(outs):
        star = of[q, NCHUNK * GC + GC * k + c]  # [NT, NG, GC]
        clique = of[q, GC * k + c]
        block = np.stack([star.reshape(EC), clique.reshape(EC)], axis=1)
        res[r * EC : (r + 1) * EC] = block
    return res[:E]


def _get_exec():
    """Build (once) the jitted sharded executable, mirroring
    concourse.bass2jax.run_bass_via_pjrt's multi-core branch."""
    if "exec" in _cached:
        return _cached["exec"]
    import jax
    from jax.sharding import Mesh, PartitionSpec
    from jax.experimental.shard_map import shard_map
    from concourse import mybir
    from concourse.bass2jax import (
        _bass_exec_p,
        install_neuronx_cc_hook,
        partition_id_tensor,
    )

    nc = _build_nc()
    _cached["nc"] = nc
    install_neuronx_cc_hook()
    assert nc.dbg_addr is None
    partition_name = nc.partition_id_tensor.name if nc.partition_id_tensor else None

    in_names, out_names, out_avals, zero_outs = [], [], [], []
    for alloc in nc.m.functions[0].allocations:
        if not isinstance(alloc, mybir.MemoryLocationSet):
            continue
        name = alloc.memorylocations[0].name
        if alloc.kind == "ExternalInput":
            if name != partition_name:
                in_names.append(name)
        elif alloc.kind == "ExternalOutput":
            out_names.append(name)
            shape = tuple(alloc.tensor_shape)
            dtype = mybir.dt.np(alloc.dtype)
            out_avals.append(jax.core.ShapedArray(shape, dtype))
            zero_outs.append(np.zeros(shape, dtype))
    n_params = len(in_names)
    n_outs = len(out_avals)
    all_names = list(in_names) + list(out_names)
    if partition_name is not None:
        all_names.append(partition_name)
    donate = tuple(range(n_params, n_params + n_outs))

    def _body(*args):
        operands = list(args)
        if partition_name is not None:
            operands.append(partition_id_tensor())
        outs = _bass_exec_p.bind(
            *operands,
            out_avals=tuple(out_avals),
            in_names=tuple(all_names),
            out_names=tuple(out_names),
            lowering_input_output_aliases=(),
            sim_require_finite=True,
            sim_require_nnan=True,
            nc=nc,
        )
        return tuple(outs)

    devices = jax.devices()[:NCORES]
    assert len(devices) == NCORES
    mesh = Mesh(np.asarray(devices), ("core",))
    in_specs = (PartitionSpec("core"),) * (n_params + n_outs)
    out_specs = (PartitionSpec("core"),) * len(out_names)
    sharded = jax.jit(
        shard_map(
            _body, mesh=mesh, in_specs=in_specs, out_specs=out_specs, check_rep=False
        ),
        donate_argnums=donate,
        keep_unused=True,
    )
    _cached["exec"] = (sharded, in_names, out_names, out_avals, zero_outs)
    return _cached["exec"]


def _run(in_maps, iters=1):
    import jax

    sharded, in_names, out_names, out_avals, zero_outs = _get_exec()
    concat_in = [
        np.concatenate([np.asarray(m[name]) for m in in_maps], axis=0)
        for name in in_names
    ]
    dev_in = [jax.device_put(a) for a in concat_in]
    times = []
    out_arrs = None
    for _ in range(max(1, iters)):
        concat_zeros = [
            np.zeros((NCORES * z.shape[0], *z.shape[1:]), z.dtype) for z in zero_outs
        ]
        import time as _time

        t0 = _time.perf_counter()
        out_arrs = sharded(*dev_in, *concat_zeros)
        jax.block_until_ready(out_arrs)
        times.append(_time.perf_counter() - t0)
    _cached["times"] = times
    return [
        {
            name: np.asarray(out_arrs[i]).reshape(NCORES, *out_avals[i].shape)[c]
            for i, name in enumerate(out_names)
        }
        for c in range(NCORES)
    ]


def kernel(node_emb, hyperedge_emb, h, X, iters=1):
    _get_exec()  # ensure vertag_shape is known before building inputs
    in_maps = _host_inputs(node_emb, hyperedge_emb, h, X)
    results = _run(in_maps, iters=iters)
    outs = [results[i]["outfin"] for i in range(NCORES)]
    return _decode(outs)

